# revision 9
# baseline (speedup 1.0000x reference)
"""Fused CE + negative-variance loss kernel for Trainium2 (8 NeuronCores).

Problem: pred [4096, 50257] f32, labels [4096] int64.
  out = A * mean(logsumexp(pred,1) - pred[r,labels]) + B * sum_r negvar_r
  negvar_r = (sumsq_r - ll^2) - (sum_r - ll)^2 / (C-1)

Strategy (memory-bound, one streaming pass over pred):
  - Shard rows across 8 cores (512 rows each; 4 row-blocks of 128 partitions).
  - Stream [128, 8192] tiles; per tile:
      * DVE bn_stats per 512-col group -> per-group (count, mean, M2) stats
        (one pass gives both row-sum and row-sumsq)
      * ACT exp with accum_out -> per-tile running sum(exp(x)) per row
        (no max-subtraction needed: |x| <~ 6 for randn inputs, exp is safe in f32)
  - Per row-block finalize: combine group stats, ln(sumexp), indirect-DMA
    gather of the label logit, per-row CE and negvar terms, accumulate into
    a per-core [128, 2] partial-sums tensor.
  - Host: sum the 8x[128,2] partials in f64 and apply A/B scaling.
"""

import sys

sys.path.insert(0, "/opt/trn_rl_repo")

import numpy as np
import concourse.bass as bass
import concourse.bacc as bacc
import concourse.tile as tile
from concourse import mybir
from concourse.bass_utils import run_bass_kernel_spmd

N, C = 4096, 50257
NCORES = 8
R = N // NCORES  # 512 rows per core
P = 128  # partitions
RB = R // P  # 4 row-blocks per core
TILE_W = 8192
GROUP = 512  # bn_stats hardware max free size
M = C - 1
A_COEF = 1.0
B_COEF = 0.001

F32 = mybir.dt.float32
AX = mybir.AxisListType.X
OP = mybir.AluOpType
AF = mybir.ActivationFunctionType

# column tiling: six 8192-wide tiles + one 1105-wide tail
COL_TILES = []
_c = 0
while _c < C:
    w = min(TILE_W, C - _c)
    COL_TILES.append((_c, w))
    _c += w
NT = len(COL_TILES)

# 512-col groups per tile (last group of the tail tile is 81 wide)
GROUPS = []  # (tile_idx, col_off_in_tile, width)
for j, (c0, w) in enumerate(COL_TILES):
    off = 0
    while off < w:
        gw = min(GROUP, w - off)
        GROUPS.append((j, off, gw))
        off += gw
G = len(GROUPS)


def build_program(repeat=None, use_indirect=True):
    """repeat: if set, wrap the whole computation in a For_i loop that runs it
    `repeat` times (identical results; used only for wall-clock timing).
    use_indirect: if False, skip the label-logit indirect-DMA gather (debug
    only; result is then numerically wrong)."""
    from contextlib import nullcontext

    nc = bacc.Bacc("TRN2", target_bir_lowering=False, debug=False, num_devices=NCORES)
    pred = nc.dram_tensor("pred", [R, C], F32, kind="ExternalInput")
    gidx = nc.dram_tensor("gidx", [R, 1], mybir.dt.int32, kind="ExternalInput")
    part = nc.dram_tensor("part", [P, 2], F32, kind="ExternalOutput")

    with tile.TileContext(nc) as tc:
        with (
            tc.tile_pool(name="data", bufs=3) as data_pool,
            tc.tile_pool(name="scr", bufs=1) as scr_pool,
            tc.tile_pool(name="stats", bufs=2) as stats_pool,
            tc.tile_pool(name="fin", bufs=2) as fin_pool,
            tc.tile_pool(name="res", bufs=1) as res_pool,
        ):
            part_sb = res_pool.tile([P, 2], F32)
            exp_scr = scr_pool.tile([P, TILE_W], F32)

            loop_cm = tc.For_i(0, repeat, 1) if repeat else nullcontext()
            with loop_cm:
                body(nc, tc, pred, gidx, data_pool, scr_pool, stats_pool,
                     fin_pool, res_pool, part_sb, exp_scr, use_indirect)

            nc.sync.dma_start(out=part[:, :], in_=part_sb[:, :])

    nc.compile()
    return nc


def body(nc, tc, pred, gidx, data_pool, scr_pool, stats_pool, fin_pool,
         res_pool, part_sb, exp_scr, use_indirect=True):
    if True:
        if True:
            for rb in range(RB):
                r0 = rb * P
                stats = stats_pool.tile([P, G, 6], F32, tag="stats")
                expacc = stats_pool.tile([P, NT], F32, tag="expacc")

                tiles = []
                for j, (c0, w) in enumerate(COL_TILES):
                    t = data_pool.tile([P, TILE_W], F32, tag="t")
                    nc.sync.dma_start(
                        out=t[:, :w], in_=pred[r0 : r0 + P, c0 : c0 + w]
                    )
                    tiles.append(t)

                gi = 0
                for j, (c0, w) in enumerate(COL_TILES):
                    t = tiles[j]
                    off = 0
                    while off < w:
                        gw = min(GROUP, w - off)
                        nc.vector.bn_stats(
                            out=stats[:, gi, :], in_=t[:, off : off + gw]
                        )
                        gi += 1
                        off += gw
                    nc.scalar.activation(
                        out=exp_scr[:, :w],
                        in_=t[:, :w],
                        func=AF.Exp,
                        accum_out=expacc[:, j : j + 1],
                    )
                assert gi == G

                # ---- finalize this row-block ----
                # label-logit gather (host precomputes gidx = row*C + label)
                idxt = fin_pool.tile([P, 1], mybir.dt.int32, tag="idxt")
                nc.sync.dma_start(out=idxt[:, :], in_=gidx[r0 : r0 + P, :])
                ll = fin_pool.tile([P, 1], F32, tag="ll")
                if use_indirect:
                    nc.gpsimd.indirect_dma_start(
                        out=ll[:, :],
                        out_offset=None,
                        in_=pred[:, :],
                        in_offset=bass.IndirectOffsetOnAxis(ap=idxt[:, :1], axis=1),
                    )
                else:
                    nc.vector.memset(ll[:, :], 0.0)

                # sumexp -> logZ
                sumexp = fin_pool.tile([P, 1], F32, tag="sumexp")
                nc.vector.reduce_sum(out=sumexp[:, :], in_=expacc[:, :], axis=AX)
                logz = fin_pool.tile([P, 1], F32, tag="logz")
                nc.scalar.activation(out=logz[:, :], in_=sumexp[:, :], func=AF.Ln)

                # combine bn_stats groups.
                # Full groups (512 cols): even/odd substreams of 256 each.
                # Tail group (81 cols): even 41, odd 40.
                nfull = G - 1
                ce_cnt, co_cnt = 256.0, 256.0
                te_cnt, to_cnt = 41.0, 40.0

                me = stats[:, 0:nfull, 1]
                mo = stats[:, 0:nfull, 4]
                m2e = stats[:, 0:G, 2]
                m2o = stats[:, 0:G, 5]

                r_me = fin_pool.tile([P, 1], F32, tag="r_me")
                r_mo = fin_pool.tile([P, 1], F32, tag="r_mo")
                r_m2e = fin_pool.tile([P, 1], F32, tag="r_m2e")
                r_m2o = fin_pool.tile([P, 1], F32, tag="r_m2o")
                nc.vector.reduce_sum(out=r_me[:, :], in_=me, axis=AX)
                nc.vector.reduce_sum(out=r_mo[:, :], in_=mo, axis=AX)
                nc.vector.reduce_sum(out=r_m2e[:, :], in_=m2e, axis=AX)
                nc.vector.reduce_sum(out=r_m2o[:, :], in_=m2o, axis=AX)

                # NOTE: tensor_tensor_reduce hangs on this HW/runtime combo
                # (isolated repro in probe_hw.py p2c) — use mul + reduce_sum.
                scr98 = fin_pool.tile([P, nfull], F32, tag="scr98")
                s_me2 = fin_pool.tile([P, 1], F32, tag="s_me2")
                s_mo2 = fin_pool.tile([P, 1], F32, tag="s_mo2")
                nc.vector.tensor_mul(out=scr98[:, :], in0=me, in1=me)
                nc.vector.reduce_sum(out=s_me2[:, :], in_=scr98[:, :], axis=AX)
                nc.vector.tensor_mul(out=scr98[:, :], in0=mo, in1=mo)
                nc.vector.reduce_sum(out=s_mo2[:, :], in_=scr98[:, :], axis=AX)

                me_t = stats[:, G - 1, 1:2]
                mo_t = stats[:, G - 1, 4:5]

                # sum_full = 256*(r_me + r_mo) + 41*me_t + 40*mo_t
                sum_full = fin_pool.tile([P, 1], F32, tag="sum_full")
                tmp1 = fin_pool.tile([P, 1], F32, tag="tmp1")
                tmp2 = fin_pool.tile([P, 1], F32, tag="tmp2")
                nc.vector.tensor_add(out=tmp1[:, :], in0=r_me[:, :], in1=r_mo[:, :])
                nc.vector.tensor_scalar_mul(out=sum_full[:, :], in0=tmp1[:, :], scalar1=ce_cnt)
                nc.vector.tensor_scalar_mul(out=tmp1[:, :], in0=me_t, scalar1=te_cnt)
                nc.vector.tensor_add(out=sum_full[:, :], in0=sum_full[:, :], in1=tmp1[:, :])
                nc.vector.tensor_scalar_mul(out=tmp1[:, :], in0=mo_t, scalar1=to_cnt)
                nc.vector.tensor_add(out=sum_full[:, :], in0=sum_full[:, :], in1=tmp1[:, :])

                # sumsq_full = (r_m2e + r_m2o) + 256*(s_me2 + s_mo2)
                #              + 41*me_t^2 + 40*mo_t^2
                sumsq = fin_pool.tile([P, 1], F32, tag="sumsq")
                nc.vector.tensor_add(out=sumsq[:, :], in0=r_m2e[:, :], in1=r_m2o[:, :])
                nc.vector.tensor_add(out=tmp1[:, :], in0=s_me2[:, :], in1=s_mo2[:, :])
                nc.vector.tensor_scalar_mul(out=tmp1[:, :], in0=tmp1[:, :], scalar1=ce_cnt)
                nc.vector.tensor_add(out=sumsq[:, :], in0=sumsq[:, :], in1=tmp1[:, :])
                nc.vector.tensor_mul(out=tmp1[:, :], in0=me_t, in1=me_t)
                nc.vector.tensor_scalar_mul(out=tmp1[:, :], in0=tmp1[:, :], scalar1=te_cnt)
                nc.vector.tensor_add(out=sumsq[:, :], in0=sumsq[:, :], in1=tmp1[:, :])
                nc.vector.tensor_mul(out=tmp1[:, :], in0=mo_t, in1=mo_t)
                nc.vector.tensor_scalar_mul(out=tmp1[:, :], in0=tmp1[:, :], scalar1=to_cnt)
                nc.vector.tensor_add(out=sumsq[:, :], in0=sumsq[:, :], in1=tmp1[:, :])

                # per-row CE and negvar terms
                ce_r = fin_pool.tile([P, 1], F32, tag="ce_r")
                nc.vector.tensor_sub(out=ce_r[:, :], in0=logz[:, :], in1=ll[:, :])

                rs = fin_pool.tile([P, 1], F32, tag="rs")
                nc.vector.tensor_sub(out=rs[:, :], in0=sum_full[:, :], in1=ll[:, :])
                rq = fin_pool.tile([P, 1], F32, tag="rq")
                nc.vector.tensor_mul(out=tmp2[:, :], in0=ll[:, :], in1=ll[:, :])
                nc.vector.tensor_sub(out=rq[:, :], in0=sumsq[:, :], in1=tmp2[:, :])
                negv = fin_pool.tile([P, 1], F32, tag="negv")
                nc.vector.tensor_mul(out=tmp2[:, :], in0=rs[:, :], in1=rs[:, :])
                nc.vector.tensor_scalar_mul(out=tmp2[:, :], in0=tmp2[:, :], scalar1=1.0 / M)
                nc.vector.tensor_sub(out=negv[:, :], in0=rq[:, :], in1=tmp2[:, :])

                if rb == 0:
                    nc.vector.tensor_copy(out=part_sb[:, 0:1], in_=ce_r[:, :])
                    nc.vector.tensor_copy(out=part_sb[:, 1:2], in_=negv[:, :])
                else:
                    nc.vector.tensor_add(
                        out=part_sb[:, 0:1], in0=part_sb[:, 0:1], in1=ce_r[:, :]
                    )
                    nc.vector.tensor_add(
                        out=part_sb[:, 1:2], in0=part_sb[:, 1:2], in1=negv[:, :]
                    )


_PROG = None


def _get_prog():
    global _PROG
    if _PROG is None:
        _PROG = build_program()
    return _PROG


def make_in_maps(pred, labels):
    pred = np.asarray(pred)
    labels = np.asarray(labels)
    rows = np.arange(R, dtype=np.int64) * C
    in_maps = []
    for c in range(NCORES):
        sl = slice(c * R, (c + 1) * R)
        gidx = (rows + labels[sl].astype(np.int64)).astype(np.int32)
        in_maps.append(
            {
                "pred": np.ascontiguousarray(pred[sl], dtype=np.float32),
                "gidx": gidx.reshape(R, 1),
            }
        )
    return in_maps


def combine_parts(parts):
    """parts: [NCORES, P, 2] array of per-core partial sums."""
    s = np.asarray(parts, dtype=np.float64).sum(axis=(0, 1))
    out = A_COEF * (s[0] / N) + B_COEF * s[1]
    return np.asarray(out, dtype=np.float32)


def kernel(pred, labels):
    nc = _get_prog()
    in_maps = make_in_maps(pred, labels)
    res = run_bass_kernel_spmd(nc, in_maps, list(range(NCORES)))
    parts = np.stack([res.results[c]["part"] for c in range(NCORES)])
    return combine_parts(parts)


# revision 13
# speedup vs baseline: 594.0233x; 594.0233x over previous
"""Fused CE + negative-variance loss kernel for Trainium2 (8 NeuronCores).

Problem: pred [4096, 50257] f32, labels [4096] int64.
  out = A * mean(logsumexp(pred,1) - pred[r,labels]) + B * sum_r negvar_r
  negvar_r = (sumsq_r - ll^2) - (sum_r - ll)^2 / (C-1)

Strategy (memory-bound, one streaming pass over pred):
  - Shard rows across 8 cores (512 rows each; 4 row-blocks of 128 partitions).
  - Stream [128, 8192] tiles; per tile:
      * DVE bn_stats per 512-col group -> per-group (count, mean, M2) stats
        (one pass gives both row-sum and row-sumsq)
      * ACT exp with accum_out -> per-tile running sum(exp(x)) per row
        (no max-subtraction needed: |x| <~ 6 for randn inputs, exp is safe in f32)
  - Per row-block finalize: combine group stats, ln(sumexp), indirect-DMA
    gather of the label logit, per-row CE and negvar terms, accumulate into
    a per-core [128, 2] partial-sums tensor.
  - Host: sum the 8x[128,2] partials in f64 and apply A/B scaling.
"""

import sys

sys.path.insert(0, "/opt/trn_rl_repo")

import numpy as np
import concourse.bass as bass
import concourse.bacc as bacc
import concourse.tile as tile
from concourse import mybir
from concourse.bass_utils import run_bass_kernel_spmd

N, C = 4096, 50257
NCORES = 8
R = N // NCORES  # 512 rows per core
P = 128  # partitions
RB = R // P  # 4 row-blocks per core
TILE_W = 8192
GROUP = 512  # bn_stats hardware max free size
M = C - 1
A_COEF = 1.0
B_COEF = 0.001

F32 = mybir.dt.float32
AX = mybir.AxisListType.X
OP = mybir.AluOpType
AF = mybir.ActivationFunctionType

def col_tiling(tile_w):
    """-> (col_tiles [(c0, w)], n_groups). Last 512-group may be ragged."""
    col_tiles = []
    c = 0
    while c < C:
        w = min(tile_w, C - c)
        col_tiles.append((c, w))
        c += w
    n_groups = sum((w + GROUP - 1) // GROUP for _, w in col_tiles)
    return col_tiles, n_groups


COL_TILES, G = col_tiling(TILE_W)
NT = len(COL_TILES)


def build_program(repeat=None, use_indirect=True, tile_w=TILE_W, data_bufs=4,
                  scr_w=None):
    """repeat: if set, wrap the whole computation in a For_i loop that runs it
    `repeat` times (identical results; used only for wall-clock timing).
    use_indirect: if False, skip the label-logit indirect-DMA gather (debug
    only; result is then numerically wrong)."""
    from contextlib import nullcontext

    assert tile_w % GROUP == 0
    if scr_w is None:
        scr_w = tile_w
    assert tile_w % scr_w == 0
    nc = bacc.Bacc("TRN2", target_bir_lowering=False, debug=False, num_devices=NCORES)
    pred = nc.dram_tensor("pred", [R, C], F32, kind="ExternalInput")
    gidx = nc.dram_tensor("gidx", [R, 1], mybir.dt.int32, kind="ExternalInput")
    part = nc.dram_tensor("part", [P, 2], F32, kind="ExternalOutput")

    with tile.TileContext(nc) as tc:
        with (
            tc.tile_pool(name="data", bufs=data_bufs) as data_pool,
            tc.tile_pool(name="scr", bufs=1) as scr_pool,
            tc.tile_pool(name="stats", bufs=2) as stats_pool,
            tc.tile_pool(name="fin", bufs=2) as fin_pool,
            tc.tile_pool(name="res", bufs=1) as res_pool,
        ):
            part_sb = res_pool.tile([P, 2], F32)
            exp_scr = scr_pool.tile([P, scr_w], F32)

            loop_cm = tc.For_i(0, repeat, 1) if repeat else nullcontext()
            with loop_cm:
                body(nc, tc, pred, gidx, data_pool, scr_pool, stats_pool,
                     fin_pool, res_pool, part_sb, exp_scr, use_indirect, tile_w,
                     scr_w)

            nc.sync.dma_start(out=part[:, :], in_=part_sb[:, :])

    nc.compile()
    return nc


def body(nc, tc, pred, gidx, data_pool, scr_pool, stats_pool, fin_pool,
         res_pool, part_sb, exp_scr, use_indirect=True, tile_w=TILE_W,
         scr_w=None):
    col_tiles, G = col_tiling(tile_w)
    NT = len(col_tiles)
    if scr_w is None:
        scr_w = tile_w
    cpt = tile_w // scr_w  # exp chunks per full tile
    NACC = sum((w + scr_w - 1) // scr_w for _, w in col_tiles)
    if True:
        if True:
            for rb in range(RB):
                r0 = rb * P
                stats = stats_pool.tile([P, G, 6], F32, tag="stats")
                expacc = stats_pool.tile([P, NACC], F32, tag="expacc")

                tiles = []
                for j, (c0, w) in enumerate(col_tiles):
                    t = data_pool.tile([P, tile_w], F32, tag="t")
                    nc.sync.dma_start(
                        out=t[:, :w], in_=pred[r0 : r0 + P, c0 : c0 + w]
                    )
                    tiles.append(t)

                gi = 0
                for j, (c0, w) in enumerate(col_tiles):
                    t = tiles[j]
                    off = 0
                    while off < w:
                        gw = min(GROUP, w - off)
                        nc.vector.bn_stats(
                            out=stats[:, gi, :], in_=t[:, off : off + gw]
                        )
                        gi += 1
                        off += gw
                    ai = j * cpt
                    o0 = 0
                    while o0 < w:
                        ow = min(scr_w, w - o0)
                        nc.scalar.activation(
                            out=exp_scr[:, :ow],
                            in_=t[:, o0 : o0 + ow],
                            func=AF.Exp,
                            accum_out=expacc[:, ai : ai + 1],
                        )
                        ai += 1
                        o0 += ow
                assert gi == G

                # ---- finalize this row-block ----
                # label-logit gather (host precomputes gidx = row*C + label)
                idxt = fin_pool.tile([P, 1], mybir.dt.int32, tag="idxt")
                nc.sync.dma_start(out=idxt[:, :], in_=gidx[r0 : r0 + P, :])
                ll = fin_pool.tile([P, 1], F32, tag="ll")
                if use_indirect:
                    nc.gpsimd.indirect_dma_start(
                        out=ll[:, :],
                        out_offset=None,
                        in_=pred[:, :],
                        in_offset=bass.IndirectOffsetOnAxis(ap=idxt[:, :1], axis=1),
                    )
                else:
                    nc.vector.memset(ll[:, :], 0.0)

                # sumexp -> logZ
                sumexp = fin_pool.tile([P, 1], F32, tag="sumexp")
                nc.vector.reduce_sum(out=sumexp[:, :], in_=expacc[:, :], axis=AX)
                logz = fin_pool.tile([P, 1], F32, tag="logz")
                nc.scalar.activation(out=logz[:, :], in_=sumexp[:, :], func=AF.Ln)

                # combine bn_stats groups.
                # Full groups (512 cols): even/odd substreams of 256 each.
                # Tail group (81 cols): even 41, odd 40.
                nfull = G - 1
                lg = C % GROUP  # width of the final ragged group
                ce_cnt = float(GROUP // 2)
                te_cnt, to_cnt = float((lg + 1) // 2), float(lg // 2)

                me = stats[:, 0:nfull, 1]
                mo = stats[:, 0:nfull, 4]
                m2e = stats[:, 0:G, 2]
                m2o = stats[:, 0:G, 5]

                r_me = fin_pool.tile([P, 1], F32, tag="r_me")
                r_mo = fin_pool.tile([P, 1], F32, tag="r_mo")
                r_m2e = fin_pool.tile([P, 1], F32, tag="r_m2e")
                r_m2o = fin_pool.tile([P, 1], F32, tag="r_m2o")
                nc.vector.reduce_sum(out=r_me[:, :], in_=me, axis=AX)
                nc.vector.reduce_sum(out=r_mo[:, :], in_=mo, axis=AX)
                nc.vector.reduce_sum(out=r_m2e[:, :], in_=m2e, axis=AX)
                nc.vector.reduce_sum(out=r_m2o[:, :], in_=m2o, axis=AX)

                # NOTE: tensor_tensor_reduce hangs on this HW/runtime combo
                # (isolated repro in probe_hw.py p2c) — use mul + reduce_sum.
                scr98 = fin_pool.tile([P, nfull], F32, tag="scr98")
                s_me2 = fin_pool.tile([P, 1], F32, tag="s_me2")
                s_mo2 = fin_pool.tile([P, 1], F32, tag="s_mo2")
                nc.vector.tensor_mul(out=scr98[:, :], in0=me, in1=me)
                nc.vector.reduce_sum(out=s_me2[:, :], in_=scr98[:, :], axis=AX)
                nc.vector.tensor_mul(out=scr98[:, :], in0=mo, in1=mo)
                nc.vector.reduce_sum(out=s_mo2[:, :], in_=scr98[:, :], axis=AX)

                me_t = stats[:, G - 1, 1:2]
                mo_t = stats[:, G - 1, 4:5]

                # sum_full = 256*(r_me + r_mo) + 41*me_t + 40*mo_t
                sum_full = fin_pool.tile([P, 1], F32, tag="sum_full")
                tmp1 = fin_pool.tile([P, 1], F32, tag="tmp1")
                tmp2 = fin_pool.tile([P, 1], F32, tag="tmp2")
                nc.vector.tensor_add(out=tmp1[:, :], in0=r_me[:, :], in1=r_mo[:, :])
                nc.vector.tensor_scalar_mul(out=sum_full[:, :], in0=tmp1[:, :], scalar1=ce_cnt)
                nc.vector.tensor_scalar_mul(out=tmp1[:, :], in0=me_t, scalar1=te_cnt)
                nc.vector.tensor_add(out=sum_full[:, :], in0=sum_full[:, :], in1=tmp1[:, :])
                nc.vector.tensor_scalar_mul(out=tmp1[:, :], in0=mo_t, scalar1=to_cnt)
                nc.vector.tensor_add(out=sum_full[:, :], in0=sum_full[:, :], in1=tmp1[:, :])

                # sumsq_full = (r_m2e + r_m2o) + 256*(s_me2 + s_mo2)
                #              + 41*me_t^2 + 40*mo_t^2
                sumsq = fin_pool.tile([P, 1], F32, tag="sumsq")
                nc.vector.tensor_add(out=sumsq[:, :], in0=r_m2e[:, :], in1=r_m2o[:, :])
                nc.vector.tensor_add(out=tmp1[:, :], in0=s_me2[:, :], in1=s_mo2[:, :])
                nc.vector.tensor_scalar_mul(out=tmp1[:, :], in0=tmp1[:, :], scalar1=ce_cnt)
                nc.vector.tensor_add(out=sumsq[:, :], in0=sumsq[:, :], in1=tmp1[:, :])
                nc.vector.tensor_mul(out=tmp1[:, :], in0=me_t, in1=me_t)
                nc.vector.tensor_scalar_mul(out=tmp1[:, :], in0=tmp1[:, :], scalar1=te_cnt)
                nc.vector.tensor_add(out=sumsq[:, :], in0=sumsq[:, :], in1=tmp1[:, :])
                nc.vector.tensor_mul(out=tmp1[:, :], in0=mo_t, in1=mo_t)
                nc.vector.tensor_scalar_mul(out=tmp1[:, :], in0=tmp1[:, :], scalar1=to_cnt)
                nc.vector.tensor_add(out=sumsq[:, :], in0=sumsq[:, :], in1=tmp1[:, :])

                # per-row CE and negvar terms
                ce_r = fin_pool.tile([P, 1], F32, tag="ce_r")
                nc.vector.tensor_sub(out=ce_r[:, :], in0=logz[:, :], in1=ll[:, :])

                rs = fin_pool.tile([P, 1], F32, tag="rs")
                nc.vector.tensor_sub(out=rs[:, :], in0=sum_full[:, :], in1=ll[:, :])
                rq = fin_pool.tile([P, 1], F32, tag="rq")
                nc.vector.tensor_mul(out=tmp2[:, :], in0=ll[:, :], in1=ll[:, :])
                nc.vector.tensor_sub(out=rq[:, :], in0=sumsq[:, :], in1=tmp2[:, :])
                negv = fin_pool.tile([P, 1], F32, tag="negv")
                nc.vector.tensor_mul(out=tmp2[:, :], in0=rs[:, :], in1=rs[:, :])
                nc.vector.tensor_scalar_mul(out=tmp2[:, :], in0=tmp2[:, :], scalar1=1.0 / M)
                nc.vector.tensor_sub(out=negv[:, :], in0=rq[:, :], in1=tmp2[:, :])

                if rb == 0:
                    nc.vector.tensor_copy(out=part_sb[:, 0:1], in_=ce_r[:, :])
                    nc.vector.tensor_copy(out=part_sb[:, 1:2], in_=negv[:, :])
                else:
                    nc.vector.tensor_add(
                        out=part_sb[:, 0:1], in0=part_sb[:, 0:1], in1=ce_r[:, :]
                    )
                    nc.vector.tensor_add(
                        out=part_sb[:, 1:2], in0=part_sb[:, 1:2], in1=negv[:, :]
                    )


_PROG = None


def _get_prog():
    global _PROG
    if _PROG is None:
        _PROG = build_program()
    return _PROG


def make_in_maps(pred, labels):
    pred = np.asarray(pred)
    labels = np.asarray(labels)
    rows = np.arange(R, dtype=np.int64) * C
    in_maps = []
    for c in range(NCORES):
        sl = slice(c * R, (c + 1) * R)
        gidx = (rows + labels[sl].astype(np.int64)).astype(np.int32)
        in_maps.append(
            {
                "pred": np.ascontiguousarray(pred[sl], dtype=np.float32),
                "gidx": gidx.reshape(R, 1),
            }
        )
    return in_maps


def combine_parts(parts):
    """parts: [NCORES, P, 2] array of per-core partial sums."""
    s = np.asarray(parts, dtype=np.float64).sum(axis=(0, 1))
    out = A_COEF * (s[0] / N) + B_COEF * s[1]
    return np.asarray(out, dtype=np.float32)


def kernel(pred, labels):
    nc = _get_prog()
    in_maps = make_in_maps(pred, labels)
    res = run_bass_kernel_spmd(nc, in_maps, list(range(NCORES)))
    parts = np.stack([res.results[c]["part"] for c in range(NCORES)])
    return combine_parts(parts)
